# revision 7
# baseline (speedup 1.0000x reference)
"""Trainium2 Bass kernel for a dense Mamba (selective-scan) block, SPMD over 8 NeuronCores.

Sharding: tensor-parallel over d_inner (2048 -> 256 channels/core), fully
pipelined over 8 time-chunks of 1024 (b,l) positions. Per chunk: in_proj
(bf16 matmul, Silu(z) fused on Act) -> depthwise causal conv via 4 diagonal
matmuls w/ PSUM accumulation + fused Silu -> x_proj partials DMAed straight
from PSUM -> per-chunk AllReduce (393KB) -> dt_proj + fused Softplus ->
selective scan per (chunk, state n): dA = Exp(dt*A[:,n]) (Act), dBx = dtx *
broadcast(B_n) (DVE bf16 2x / partial GpSimd offload), hardware
tensor_tensor_scan (fp32 state), y_n = h * broadcast(C_n), n-reduction via
identity-matmul PSUM accumulation -> gate = (xs*D + psum) via
scalar_tensor_tensor reading PSUM + silu(z) multiply -> per-chunk AllToAll
(512KB) -> out_proj on 128 t-rows with resident weights -> interleaved
per-core output blocks, host re-interleave.

Shapes hardcoded for: B=2, L=4096, d_model=1024, d_inner=2048, d_state=16,
d_conv=4, dt_rank=64, f32 I/O.
"""
import numpy as np
import ml_dtypes
from contextlib import ExitStack

import concourse.bass as bass
import concourse.bacc as bacc
import concourse.tile as tile
from concourse import mybir
from concourse import bass_utils

BF = ml_dtypes.bfloat16
F32 = mybir.dt.float32
BF16 = mybir.dt.bfloat16

NCORES = 8
B, L, DM = 2, 4096, 1024
DI, DS, DC, DTR = 2048, 16, 4, 64
DL = DI // NCORES          # 256 local channels
NDH = DL // 128            # 2 d-half tiles
T = B * L                  # 8192 flattened (b, l)
TCC = 1024                 # pipeline chunk
NTCB = T // TCC            # 8
TCA = 512                  # in_proj sub-chunk
GP_NS = ()         # dBx muls offloaded to gpsimd for these n

_cached = {}


def _build():
    nc = bacc.Bacc("TRN2", target_bir_lowering=False, num_devices=NCORES)

    # ---- I/O -------------------------------------------------------------
    d_hT = nc.dram_tensor("hT", (DM, T), BF16, kind="ExternalInput")
    d_wxzT = nc.dram_tensor("wxzT", (DM, 2 * DL), BF16, kind="ExternalInput")
    d_cdiag = nc.dram_tensor("cdiag", (DC, NDH, 128, 128), BF16, kind="ExternalInput")
    d_cbrow = nc.dram_tensor("cbrow", (1, NDH, 128), BF16, kind="ExternalInput")
    d_xprojT = nc.dram_tensor("xprojT", (NDH, 128, DTR + 2 * DS), BF16, kind="ExternalInput")
    d_dtwT = nc.dram_tensor("dtwT", (DTR, DL), BF16, kind="ExternalInput")
    d_dtb = nc.dram_tensor("dtb", (NDH, 128, 1), F32, kind="ExternalInput")
    d_aneg = nc.dram_tensor("aneg", (NDH, 128, DS), F32, kind="ExternalInput")
    d_dvec = nc.dram_tensor("dvec", (NDH, 128, 1), F32, kind="ExternalInput")
    d_woutT = nc.dram_tensor("woutT", (2 * NCORES, 128, DM), BF16, kind="ExternalInput")
    d_ident = nc.dram_tensor("ident", (128, 128), BF16, kind="ExternalInput")
    d_out = nc.dram_tensor("out_slice", (NTCB * 128, DM), F32, kind="ExternalOutput")

    # ---- internal DRAM ---------------------------------------------------
    d_xdp = nc.dram_tensor("xdp", (NTCB, DTR + 2 * DS, TCC), F32, kind="Internal")
    d_xd = nc.dram_tensor("xd", (NTCB, DTR + 2 * DS, TCC), F32, kind="Internal",
                          addr_space="Shared")
    d_bc = nc.dram_tensor("bcrows", (2 * DS, T), BF16, kind="Internal")
    d_a2ai = nc.dram_tensor("a2ai", (NTCB, NCORES, DL, 128), BF16, kind="Internal")
    d_a2ao = nc.dram_tensor("a2ao", (NTCB, NCORES, DL, 128), BF16, kind="Internal")

    groups = [list(range(NCORES))]
    Ident = mybir.ActivationFunctionType.Identity
    Tanh = mybir.ActivationFunctionType.Tanh
    Lnf = mybir.ActivationFunctionType.Ln
    Expf = mybir.ActivationFunctionType.Exp
    MUL, ADD = mybir.AluOpType.mult, mybir.AluOpType.add

    with tile.TileContext(nc) as tc, ExitStack() as ctx:
        consts = ctx.enter_context(tc.tile_pool(name="consts", bufs=1))
        arena = ctx.enter_context(tc.tile_pool(name="arena", bufs=2))
        work = ctx.enter_context(tc.tile_pool(name="work", bufs=2))
        work2 = ctx.enter_context(tc.tile_pool(name="work2", bufs=2))
        bcp = ctx.enter_context(tc.tile_pool(name="bcp", bufs=4))
        psA = ctx.enter_context(tc.tile_pool(name="psA", bufs=2, space="PSUM"))
        psY = ctx.enter_context(tc.tile_pool(name="psY", bufs=4, space="PSUM"))
        psE = ctx.enter_context(tc.tile_pool(name="psE", bufs=2, space="PSUM"))

        # ---- constants ----------------------------------------------------
        wxz = consts.tile([128, 8, 2 * DL], BF16, tag="wxz")
        for k8 in range(8):
            nc.sync.dma_start(
                out=wxz[:, k8, :],
                in_=bass.AP(tensor=d_wxzT[:, :].tensor, offset=k8 * 128 * 2 * DL,
                            ap=[[2 * DL, 128], [1, 2 * DL]]))
        cdg = consts.tile([128, DC, NDH, 128], BF16, tag="cdg")
        nc.sync.dma_start(
            out=cdg, in_=bass.AP(tensor=d_cdiag[:, :, :, :].tensor, offset=0,
                                 ap=[[128, 128], [NDH * 128 * 128, DC], [128 * 128, NDH], [1, 128]]))
        cbrow = consts.tile([1, NDH, 128], BF16, tag="cbrow")
        nc.sync.dma_start(out=cbrow, in_=d_cbrow[:, :, :])
        onesr = consts.tile([1, TCA], BF16, tag="onesr")
        nc.gpsimd.memset(onesr, 1.0)
        xprj = consts.tile([128, NDH, DTR + 2 * DS], BF16, tag="xprj")
        nc.sync.dma_start(out=xprj, in_=d_xprojT[:, :, :].rearrange("h p m -> p h m"))
        dtw = consts.tile([DTR, DL], BF16, tag="dtw")
        nc.sync.dma_start(out=dtw, in_=d_dtwT[:, :])
        dtb = consts.tile([128, NDH, 1], F32, tag="dtb")
        nc.sync.dma_start(out=dtb, in_=d_dtb[:, :, :].rearrange("h p one -> p h one"))
        aneg = consts.tile([128, NDH, DS], F32, tag="aneg")
        nc.sync.dma_start(out=aneg, in_=d_aneg[:, :, :].rearrange("h p n -> p h n"))
        dvec = consts.tile([128, NDH, 1], F32, tag="dvec")
        nc.sync.dma_start(out=dvec, in_=d_dvec[:, :, :].rearrange("h p one -> p h one"))
        wout = consts.tile([128, 2 * NCORES, DM], BF16, tag="wout")
        for kt in range(2 * NCORES):
            nc.sync.dma_start(
                out=wout[:, kt, :],
                in_=bass.AP(tensor=d_woutT[:, :, :].tensor, offset=kt * 128 * DM,
                            ap=[[DM, 128], [1, DM]]))
        ident = consts.tile([128, 128], BF16, tag="ident")
        nc.sync.dma_start(out=ident, in_=d_ident[:, :])
        carry = consts.tile([128, NDH, DS], F32, tag="carry")

        xpad_hist = [None] * (NTCB + 1)
        zsil_hist = [None] * NTCB
        xs_hist = [None] * NTCB
        dts_hist = [None] * NTCB
        dtx_hist = [None] * NTCB

        def emitA(j):
            t0 = j * TCC
            xpad_prev = xpad_hist[j - 1] if j > 0 else None
            # ---- A: in_proj (+ fused Silu for z) -------------------------
            xpad = arena.tile([128, NDH, 1027], BF16, tag="xpad")
            zsil = arena.tile([128, NDH, TCC], BF16, tag="zsil")
            xs = arena.tile([128, NDH, TCC], BF16, tag="xs")
            if j % (L // TCC) == 0:
                nc.vector.memset(xpad[:, 0, 0:3], 0.0)
                nc.vector.memset(xpad[:, 1, 0:3], 0.0)
            else:
                nc.vector.tensor_copy(xpad[:, 0, 0:3], xpad_prev[:, 0, 1024:1027])
                nc.vector.tensor_copy(xpad[:, 1, 0:3], xpad_prev[:, 1, 1024:1027])
            for s in range(2):
                ts0 = t0 + s * TCA
                ht = work.tile([128, 8, TCA], BF16, tag="ht")
                nc.sync.dma_start(
                    out=ht,
                    in_=bass.AP(tensor=d_hT[:, :].tensor, offset=ts0,
                                ap=[[T, 128], [128 * T, 8], [1, TCA]]))
                for m in range(4):  # 0,1: x halves; 2,3: z halves
                    pxz = psA.tile([128, TCA], F32, tag="ps")
                    for k in range(8):
                        nc.tensor.matmul(pxz, lhsT=wxz[:, k, m * 128:(m + 1) * 128],
                                         rhs=ht[:, k, :], start=(k == 0), stop=(k == 7))
                    if m < 2:
                        nc.scalar.activation(
                            xpad[:, m, 3 + s * TCA: 3 + s * TCA + TCA], pxz,
                            Ident, bias=0.0, scale=1.0)
                    else:
                        th = work.tile([128, TCA], BF16, tag="th")
                        nc.scalar.activation(th, pxz, Tanh, bias=0.0, scale=0.5)
                        nc.vector.scalar_tensor_tensor(
                            out=zsil[:, m - 2, s * TCA: s * TCA + TCA],
                            in0=th, scalar=1.0, in1=pxz, op0=ADD, op1=MUL)
            # ---- conv (4 diag matmuls) + fused Silu ----------------------
            for h in range(NDH):
                for s in range(2):
                    l0s = s * TCA
                    pc = psA.tile([128, TCA], F32, tag="ps")
                    nc.tensor.matmul(pc, lhsT=cbrow[:, h, :], rhs=onesr[:, :],
                                     start=True, stop=False)
                    for jj in range(DC):
                        nc.tensor.matmul(pc, lhsT=cdg[:, jj, h, :],
                                         rhs=xpad[:, h, l0s + jj: l0s + jj + TCA],
                                         start=False, stop=(jj == DC - 1))
                    th = work.tile([128, TCA], BF16, tag="th")
                    nc.scalar.activation(th, pc, Tanh, bias=0.0, scale=0.5)
                    nc.vector.scalar_tensor_tensor(
                        out=xs[:, h, l0s:l0s + TCA],
                        in0=th, scalar=1.0, in1=pc, op0=ADD, op1=MUL)
            # ---- x_proj partial -> DMA straight from PSUM ----------------
            for s in range(2):
                pxp = psA.tile([128, TCA], F32, tag="ps")
                for h in range(NDH):
                    nc.tensor.matmul(pxp[0:96, :], lhsT=xprj[:, h, :],
                                     rhs=xs[:, h, s * TCA:(s + 1) * TCA],
                                     start=(h == 0), stop=(h == NDH - 1))
                xpt = work.tile([96, TCA], F32, tag="xpt")
                nc.scalar.activation(xpt, pxp[0:96, :], Ident, bias=0.0, scale=1.0)
                nc.sync.dma_start(out=d_xdp[j, :, s * TCA:(s + 1) * TCA], in_=xpt)
            # ---- AllReduce of this chunk's x_dbl partials ----------------
            nc.gpsimd.collective_compute(
                kind="AllReduce", op=mybir.AluOpType.add, replica_groups=groups,
                ins=[d_xdp[j, :, :]], outs=[d_xd[j, :, :]])
            xpad_hist[j] = xpad
            zsil_hist[j] = zsil
            xs_hist[j] = xs

        def emitB(j):
            t0 = j * TCC
            xs = xs_hist[j]
            # ---- B: dt_proj + fused Softplus; dtx; B/C rows --------------
            dts = arena.tile([128, NDH, TCC], BF16, tag="dts")
            dtx = arena.tile([128, NDH, TCC], BF16, tag="dtx")
            xdt = work.tile([96, TCC], F32, tag="xdt")
            nc.sync.dma_start(out=xdt, in_=d_xd[j, :, :])
            xdb = work.tile([96, TCC], BF16, tag="xdb")
            nc.vector.tensor_copy(xdb, xdt)
            nc.sync.dma_start(out=d_bc[:, t0:t0 + TCC], in_=xdb[DTR:DTR + 2 * DS, :])
            for h in range(NDH):
                for s in range(2):
                    pdt = psA.tile([128, TCA], F32, tag="ps")
                    nc.tensor.matmul(pdt, lhsT=dtw[:, h * 128:(h + 1) * 128],
                                     rhs=xdb[0:DTR, s * TCA:(s + 1) * TCA],
                                     start=True, stop=True)
                    spe = work.tile([128, TCA], F32, tag="spe")
                    nc.scalar.activation(spe, pdt, Expf,
                                         bias=dtb[:, h, 0:1], scale=1.0)
                    nc.scalar.activation(dts[:, h, s * TCA:(s + 1) * TCA], spe,
                                         Lnf, bias=1.0, scale=1.0)
                nc.vector.tensor_mul(dtx[:, h, :], dts[:, h, :], xs[:, h, :])
            dts_hist[j] = dts
            dtx_hist[j] = dtx

        def emitC(j):
            t0 = j * TCC
            xs, zsil = xs_hist[j], zsil_hist[j]
            dts, dtx = dts_hist[j], dtx_hist[j]
            # ---- C: selective scan over n --------------------------------
            pys = [[psY.tile([128, 512], F32, tag="py", name=f"pys_{j}_{h2}_{q2}")
                    for q2 in range(2)] for h2 in range(NDH)]
            for n in range(DS):
                bbc = bcp.tile([128, TCC], BF16, tag="bbc")
                cbc = bcp.tile([128, TCC], BF16, tag="cbc")
                for hv in range(2):
                    nc.sync.dma_start(
                        out=bbc[:, hv * 512:(hv + 1) * 512],
                        in_=bass.AP(tensor=d_bc[:, :].tensor,
                                    offset=n * T + t0 + hv * 512,
                                    ap=[[0, 128], [1, 512]]))
                    nc.sync.dma_start(
                        out=cbc[:, hv * 512:(hv + 1) * 512],
                        in_=bass.AP(tensor=d_bc[:, :].tensor,
                                    offset=(DS + n) * T + t0 + hv * 512,
                                    ap=[[0, 128], [1, 512]]))
                for h in range(NDH):
                    dA = work2.tile([128, TCC], F32, tag="dA")
                    nc.scalar.activation(dA, dts[:, h, :], Expf,
                                         bias=0.0, scale=aneg[:, h, n:n + 1])
                    dBx = work2.tile([128, TCC], BF16, tag="dBx")
                    if n in GP_NS:
                        nc.gpsimd.tensor_mul(dBx, dtx[:, h, :], bbc)
                    else:
                        nc.vector.tensor_mul(dBx, dtx[:, h, :], bbc)
                    hts = work2.tile([128, TCC], BF16, tag="hts")
                    init = 0.0 if (t0 % L == 0) else carry[:, h, n:n + 1]
                    nc.vector.tensor_tensor_scan(
                        out=hts, data0=dA, data1=dBx, initial=init,
                        op0=MUL, op1=ADD)
                    if (t0 + TCC) % L != 0:
                        nc.vector.tensor_copy(carry[:, h, n:n + 1], hts[:, TCC - 1:TCC])
                    yp = work2.tile([128, TCC], BF16, tag="yp")
                    nc.vector.tensor_mul(yp, hts, cbc)
                    for qq in range(2):
                        nc.tensor.matmul(pys[h][qq], lhsT=ident,
                                         rhs=yp[:, qq * 512:(qq + 1) * 512],
                                         start=(n == 0), stop=(n == DS - 1))
            # ---- gate + AllToAll input -----------------------------------
            for h in range(NDH):
                for qq in range(2):
                    g1 = work2.tile([128, 512], BF16, tag="g1")
                    nc.vector.scalar_tensor_tensor(
                        out=g1, in0=xs[:, h, qq * 512:(qq + 1) * 512],
                        scalar=dvec[:, h, 0:1], in1=pys[h][qq],
                        op0=MUL, op1=ADD)
                    ys = work2.tile([128, 512], BF16, tag="ys")
                    nc.vector.tensor_mul(ys, g1, zsil[:, h, qq * 512:(qq + 1) * 512])
                    for i2 in range(4):
                        i = qq * 4 + i2
                        nc.sync.dma_start(
                            out=d_a2ai[j, i, h * 128:(h + 1) * 128, :],
                            in_=ys[:, i2 * 128:(i2 + 1) * 128])
            # ---- AllToAll of this chunk ----------------------------------
            nc.gpsimd.collective_compute(
                kind="AllToAll", op=mybir.AluOpType.bypass, replica_groups=groups,
                ins=[d_a2ai[j, :, :, :]], outs=[d_a2ao[j, :, :, :]])
        def emitE(j):
            # ---- E: out_proj on this core's 128 t-rows of chunk j --------
            ykt = work.tile([128, NCORES, NDH, 128], BF16, tag="ykt")
            nc.sync.dma_start(
                out=ykt, in_=bass.AP(tensor=d_a2ao[:, :, :, :].tensor,
                                     offset=j * NCORES * DL * 128,
                                     ap=[[128, 128], [DL * 128, NCORES],
                                         [128 * 128, NDH], [1, 128]]))
            for fh in range(2):
                pos = psE.tile([128, 512], F32, tag="pe")
                for kt in range(2 * NCORES):
                    i, h = kt // 2, kt % 2
                    nc.tensor.matmul(pos, lhsT=ykt[:, i, h, :],
                                     rhs=wout[:, kt, fh * 512:(fh + 1) * 512],
                                     start=(kt == 0), stop=(kt == 2 * NCORES - 1))
                oc = work.tile([128, 512], F32, tag="oc")
                nc.scalar.activation(oc, pos, Ident, bias=0.0, scale=1.0)
                nc.sync.dma_start(
                    out=d_out[j * 128:(j + 1) * 128, fh * 512:(fh + 1) * 512],
                    in_=oc)

        emitA(0)
        for j in range(NTCB):
            emitB(j)
            if j + 1 < NTCB:
                emitA(j + 1)
            emitC(j)
            emitE(j)

    nc.compile()
    return nc


def _host_prep(inputs):
    """Per-core input maps from full inputs (layout prep + bf16 casts only)."""
    hs = np.asarray(inputs["hidden_states"], np.float32)
    wxz = np.asarray(inputs["in_proj_w"], np.float32)
    cw = np.asarray(inputs["conv_w"], np.float32)
    cb = np.asarray(inputs["conv_b"], np.float32)
    xpw = np.asarray(inputs["x_proj_w"], np.float32)
    dpw = np.asarray(inputs["dt_proj_w"], np.float32)
    dpb = np.asarray(inputs["dt_proj_b"], np.float32)
    alog = np.asarray(inputs["A_log"], np.float32)
    dv = np.asarray(inputs["D"], np.float32)
    wo = np.asarray(inputs["out_proj_w"], np.float32)

    hT = np.ascontiguousarray(hs.reshape(T, DM).T).astype(BF)
    woutT = (np.ascontiguousarray(wo.T) * 0.5).reshape(2 * NCORES, 128, DM).astype(BF)
    ident = (np.eye(128, dtype=np.float32) * 0.5).astype(BF)

    in_maps = []
    for i in range(NCORES):
        lo = i * DL
        sl = slice(lo, lo + DL)
        wxzT = np.ascontiguousarray(
            np.concatenate([wxz[sl], wxz[DI + lo:DI + lo + DL]], axis=0).T).astype(BF)
        cdiag = np.zeros((DC, NDH, 128, 128), np.float32)
        for jc in range(DC):
            for h in range(NDH):
                np.fill_diagonal(cdiag[jc, h], cw[lo + h * 128:lo + (h + 1) * 128, jc])
        in_maps.append({
            "hT": hT,
            "wxzT": wxzT,
            "cdiag": cdiag.astype(BF),
            "cbrow": cb[sl].reshape(1, NDH, 128).astype(BF),
            "xprojT": (np.ascontiguousarray(xpw[:, sl].T) * 0.5).reshape(NDH, 128, 96).astype(BF),
            "dtwT": np.ascontiguousarray(dpw[sl].T).astype(BF),
            "dtb": dpb[sl].reshape(NDH, 128, 1),
            "aneg": (-np.exp(alog[sl])).reshape(NDH, 128, DS).astype(np.float32),
            "dvec": (dv[sl] * 0.5).reshape(NDH, 128, 1),
            "woutT": woutT,
            "ident": ident,
        })
    return in_maps


def _run(inputs, trace=False, **kw):
    if "nc" not in _cached:
        _cached["nc"] = _build()
    nc = _cached["nc"]
    in_maps = _host_prep(inputs)
    res = bass_utils.run_bass_kernel_spmd(
        nc, in_maps, core_ids=list(range(NCORES)), trace=trace, **kw)
    # core i's out_slice rows [j*128:(j+1)*128] are global t rows
    # [j*1024 + i*128 : j*1024 + (i+1)*128]
    full = np.zeros((T, DM), np.float32)
    for i in range(NCORES):
        o = res.results[i]["out_slice"]
        for j in range(NTCB):
            full[j * TCC + i * 128: j * TCC + (i + 1) * 128] = \
                o[j * 128:(j + 1) * 128]
    return full.reshape(B, L, DM), res


def kernel(**inputs):
    out, _ = _run(inputs, trace=False)
    return out


# revision 8
# speedup vs baseline: 1.0258x; 1.0258x over previous
"""Trainium2 Bass kernel for a dense Mamba (selective-scan) block, SPMD over 8 NeuronCores.

Sharding: tensor-parallel over d_inner (2048 -> 256 channels/core), fully
pipelined over 8 time-chunks of 1024 (b,l) positions. Per chunk: in_proj
(bf16 matmul, Silu(z) fused on Act) -> depthwise causal conv via 4 diagonal
matmuls w/ PSUM accumulation + fused Silu -> x_proj partials DMAed straight
from PSUM -> per-chunk AllReduce (393KB) -> dt_proj + fused Softplus ->
selective scan per (chunk, state n): dA = Exp(dt*A[:,n]) (Act), dBx = dtx *
broadcast(B_n) (DVE bf16 2x / partial GpSimd offload), hardware
tensor_tensor_scan (fp32 state), y_n = h * broadcast(C_n), n-reduction via
identity-matmul PSUM accumulation -> gate = (xs*D + psum) via
scalar_tensor_tensor reading PSUM + silu(z) multiply -> per-chunk AllToAll
(512KB) -> out_proj on 128 t-rows with resident weights -> interleaved
per-core output blocks, host re-interleave.

Shapes hardcoded for: B=2, L=4096, d_model=1024, d_inner=2048, d_state=16,
d_conv=4, dt_rank=64, f32 I/O.
"""
import numpy as np
import ml_dtypes
from contextlib import ExitStack

import concourse.bass as bass
import concourse.bacc as bacc
import concourse.tile as tile
from concourse import mybir
from concourse import bass_utils

BF = ml_dtypes.bfloat16
F32 = mybir.dt.float32
BF16 = mybir.dt.bfloat16

NCORES = 8
B, L, DM = 2, 4096, 1024
DI, DS, DC, DTR = 2048, 16, 4, 64
DL = DI // NCORES          # 256 local channels
NDH = DL // 128            # 2 d-half tiles
T = B * L                  # 8192 flattened (b, l)
TCC = 1024                 # pipeline chunk
NTCB = T // TCC            # 8
TCA = 512                  # in_proj sub-chunk
GP_NS = ()         # dBx muls offloaded to gpsimd for these n

_cached = {}


def _build():
    nc = bacc.Bacc("TRN2", target_bir_lowering=False, num_devices=NCORES)

    # ---- I/O -------------------------------------------------------------
    d_hT = nc.dram_tensor("hT", (DM, T), BF16, kind="ExternalInput")
    d_wxzT = nc.dram_tensor("wxzT", (DM, 2 * DL), BF16, kind="ExternalInput")
    d_cdiag = nc.dram_tensor("cdiag", (DC, NDH, 128, 128), BF16, kind="ExternalInput")
    d_cbrow = nc.dram_tensor("cbrow", (1, NDH, 128), BF16, kind="ExternalInput")
    d_xprojT = nc.dram_tensor("xprojT", (NDH, 128, DTR + 2 * DS), BF16, kind="ExternalInput")
    d_dtwT = nc.dram_tensor("dtwT", (DTR, DL), BF16, kind="ExternalInput")
    d_dtb = nc.dram_tensor("dtb", (NDH, 128, 1), F32, kind="ExternalInput")
    d_aneg = nc.dram_tensor("aneg", (NDH, 128, DS), F32, kind="ExternalInput")
    d_dvec = nc.dram_tensor("dvec", (NDH, 128, 1), F32, kind="ExternalInput")
    d_woutT = nc.dram_tensor("woutT", (2 * NCORES, 128, DM), BF16, kind="ExternalInput")
    d_ident = nc.dram_tensor("ident", (128, 128), BF16, kind="ExternalInput")
    d_out = nc.dram_tensor("out_slice", (NTCB * 128, DM), F32, kind="ExternalOutput")

    # ---- internal DRAM ---------------------------------------------------
    d_xdp = nc.dram_tensor("xdp", (NTCB, DTR + 2 * DS, TCC), F32, kind="Internal")
    d_xd = nc.dram_tensor("xd", (NTCB, DTR + 2 * DS, TCC), F32, kind="Internal",
                          addr_space="Shared")
    d_bc = nc.dram_tensor("bcrows", (2 * DS, T), BF16, kind="Internal")
    d_a2ai = nc.dram_tensor("a2ai", (NTCB, NCORES, DL, 128), BF16, kind="Internal")
    d_a2ao = nc.dram_tensor("a2ao", (NTCB, NCORES, DL, 128), BF16, kind="Internal")

    groups = [list(range(NCORES))]
    Ident = mybir.ActivationFunctionType.Identity
    Tanh = mybir.ActivationFunctionType.Tanh
    Lnf = mybir.ActivationFunctionType.Ln
    Expf = mybir.ActivationFunctionType.Exp
    MUL, ADD = mybir.AluOpType.mult, mybir.AluOpType.add

    with tile.TileContext(nc) as tc, ExitStack() as ctx:
        consts = ctx.enter_context(tc.tile_pool(name="consts", bufs=1))
        arena = ctx.enter_context(tc.tile_pool(name="arena", bufs=2))
        work = ctx.enter_context(tc.tile_pool(name="work", bufs=2))
        work2 = ctx.enter_context(tc.tile_pool(name="work2", bufs=3))
        bcp = ctx.enter_context(tc.tile_pool(name="bcp", bufs=4))
        psA = ctx.enter_context(tc.tile_pool(name="psA", bufs=2, space="PSUM"))
        psY = ctx.enter_context(tc.tile_pool(name="psY", bufs=4, space="PSUM"))
        psE = ctx.enter_context(tc.tile_pool(name="psE", bufs=2, space="PSUM"))

        # ---- constants ----------------------------------------------------
        wxz = consts.tile([128, 8, 2 * DL], BF16, tag="wxz")
        for k8 in range(8):
            nc.sync.dma_start(
                out=wxz[:, k8, :],
                in_=bass.AP(tensor=d_wxzT[:, :].tensor, offset=k8 * 128 * 2 * DL,
                            ap=[[2 * DL, 128], [1, 2 * DL]]))
        cdg = consts.tile([128, DC, NDH, 128], BF16, tag="cdg")
        nc.sync.dma_start(
            out=cdg, in_=bass.AP(tensor=d_cdiag[:, :, :, :].tensor, offset=0,
                                 ap=[[128, 128], [NDH * 128 * 128, DC], [128 * 128, NDH], [1, 128]]))
        cbrow = consts.tile([1, NDH, 128], BF16, tag="cbrow")
        nc.sync.dma_start(out=cbrow, in_=d_cbrow[:, :, :])
        onesr = consts.tile([1, TCA], BF16, tag="onesr")
        nc.gpsimd.memset(onesr, 1.0)
        xprj = consts.tile([128, NDH, DTR + 2 * DS], BF16, tag="xprj")
        nc.sync.dma_start(out=xprj, in_=d_xprojT[:, :, :].rearrange("h p m -> p h m"))
        dtw = consts.tile([DTR, DL], BF16, tag="dtw")
        nc.sync.dma_start(out=dtw, in_=d_dtwT[:, :])
        dtb = consts.tile([128, NDH, 1], F32, tag="dtb")
        nc.sync.dma_start(out=dtb, in_=d_dtb[:, :, :].rearrange("h p one -> p h one"))
        aneg = consts.tile([128, NDH, DS], F32, tag="aneg")
        nc.sync.dma_start(out=aneg, in_=d_aneg[:, :, :].rearrange("h p n -> p h n"))
        dvec = consts.tile([128, NDH, 1], F32, tag="dvec")
        nc.sync.dma_start(out=dvec, in_=d_dvec[:, :, :].rearrange("h p one -> p h one"))
        wout = consts.tile([128, 2 * NCORES, DM], BF16, tag="wout")
        for kt in range(2 * NCORES):
            nc.sync.dma_start(
                out=wout[:, kt, :],
                in_=bass.AP(tensor=d_woutT[:, :, :].tensor, offset=kt * 128 * DM,
                            ap=[[DM, 128], [1, DM]]))
        ident = consts.tile([128, 128], BF16, tag="ident")
        nc.sync.dma_start(out=ident, in_=d_ident[:, :])
        carry = consts.tile([128, NDH, DS], F32, tag="carry")

        xpad_hist = [None] * (NTCB + 1)
        zsil_hist = [None] * NTCB
        xs_hist = [None] * NTCB
        dts_hist = [None] * NTCB
        dtx_hist = [None] * NTCB

        def emitA(j):
            t0 = j * TCC
            xpad_prev = xpad_hist[j - 1] if j > 0 else None
            # ---- A: in_proj (+ fused Silu for z) -------------------------
            xpad = arena.tile([128, NDH, 1027], BF16, tag="xpad")
            zsil = arena.tile([128, NDH, TCC], BF16, tag="zsil")
            xs = arena.tile([128, NDH, TCC], BF16, tag="xs")
            if j % (L // TCC) == 0:
                nc.vector.memset(xpad[:, 0, 0:3], 0.0)
                nc.vector.memset(xpad[:, 1, 0:3], 0.0)
            else:
                nc.vector.tensor_copy(xpad[:, 0, 0:3], xpad_prev[:, 0, 1024:1027])
                nc.vector.tensor_copy(xpad[:, 1, 0:3], xpad_prev[:, 1, 1024:1027])
            for s in range(2):
                ts0 = t0 + s * TCA
                ht = work.tile([128, 8, TCA], BF16, tag="ht")
                nc.sync.dma_start(
                    out=ht,
                    in_=bass.AP(tensor=d_hT[:, :].tensor, offset=ts0,
                                ap=[[T, 128], [128 * T, 8], [1, TCA]]))
                for m in range(4):  # 0,1: x halves; 2,3: z halves
                    pxz = psA.tile([128, TCA], F32, tag="ps")
                    for k in range(8):
                        nc.tensor.matmul(pxz, lhsT=wxz[:, k, m * 128:(m + 1) * 128],
                                         rhs=ht[:, k, :], start=(k == 0), stop=(k == 7))
                    if m < 2:
                        nc.scalar.activation(
                            xpad[:, m, 3 + s * TCA: 3 + s * TCA + TCA], pxz,
                            Ident, bias=0.0, scale=1.0)
                    else:
                        th = work.tile([128, TCA], BF16, tag="th")
                        nc.scalar.activation(th, pxz, Tanh, bias=0.0, scale=0.5)
                        nc.vector.scalar_tensor_tensor(
                            out=zsil[:, m - 2, s * TCA: s * TCA + TCA],
                            in0=th, scalar=1.0, in1=pxz, op0=ADD, op1=MUL)
            # ---- conv (4 diag matmuls) + fused Silu ----------------------
            for h in range(NDH):
                for s in range(2):
                    l0s = s * TCA
                    pc = psA.tile([128, TCA], F32, tag="ps")
                    nc.tensor.matmul(pc, lhsT=cbrow[:, h, :], rhs=onesr[:, :],
                                     start=True, stop=False)
                    for jj in range(DC):
                        nc.tensor.matmul(pc, lhsT=cdg[:, jj, h, :],
                                         rhs=xpad[:, h, l0s + jj: l0s + jj + TCA],
                                         start=False, stop=(jj == DC - 1))
                    th = work.tile([128, TCA], BF16, tag="th")
                    nc.scalar.activation(th, pc, Tanh, bias=0.0, scale=0.5)
                    nc.vector.scalar_tensor_tensor(
                        out=xs[:, h, l0s:l0s + TCA],
                        in0=th, scalar=1.0, in1=pc, op0=ADD, op1=MUL)
            # ---- x_proj partial -> DMA straight from PSUM ----------------
            for s in range(2):
                pxp = psA.tile([128, TCA], F32, tag="ps")
                for h in range(NDH):
                    nc.tensor.matmul(pxp[0:96, :], lhsT=xprj[:, h, :],
                                     rhs=xs[:, h, s * TCA:(s + 1) * TCA],
                                     start=(h == 0), stop=(h == NDH - 1))
                xpt = work.tile([96, TCA], F32, tag="xpt")
                nc.scalar.activation(xpt, pxp[0:96, :], Ident, bias=0.0, scale=1.0)
                nc.sync.dma_start(out=d_xdp[j, :, s * TCA:(s + 1) * TCA], in_=xpt)
            # ---- AllReduce of this chunk's x_dbl partials ----------------
            nc.gpsimd.collective_compute(
                kind="AllReduce", op=mybir.AluOpType.add, replica_groups=groups,
                ins=[d_xdp[j, :, :]], outs=[d_xd[j, :, :]])
            xpad_hist[j] = xpad
            zsil_hist[j] = zsil
            xs_hist[j] = xs

        def emitB(j):
            t0 = j * TCC
            xs = xs_hist[j]
            # ---- B: dt_proj + fused Softplus; dtx; B/C rows --------------
            dts = arena.tile([128, NDH, TCC], BF16, tag="dts")
            dtx = arena.tile([128, NDH, TCC], BF16, tag="dtx")
            xdt = work.tile([96, TCC], F32, tag="xdt")
            nc.sync.dma_start(out=xdt, in_=d_xd[j, :, :])
            xdb = work.tile([96, TCC], BF16, tag="xdb")
            nc.vector.tensor_copy(xdb, xdt)
            nc.sync.dma_start(out=d_bc[:, t0:t0 + TCC], in_=xdb[DTR:DTR + 2 * DS, :])
            for h in range(NDH):
                for s in range(2):
                    pdt = psA.tile([128, TCA], F32, tag="ps")
                    nc.tensor.matmul(pdt, lhsT=dtw[:, h * 128:(h + 1) * 128],
                                     rhs=xdb[0:DTR, s * TCA:(s + 1) * TCA],
                                     start=True, stop=True)
                    spe = work.tile([128, TCA], F32, tag="spe")
                    nc.scalar.activation(spe, pdt, Expf,
                                         bias=dtb[:, h, 0:1], scale=1.0)
                    nc.scalar.activation(dts[:, h, s * TCA:(s + 1) * TCA], spe,
                                         Lnf, bias=1.0, scale=1.0)
                nc.vector.tensor_mul(dtx[:, h, :], dts[:, h, :], xs[:, h, :])
            dts_hist[j] = dts
            dtx_hist[j] = dtx

        def emitC(j):
            t0 = j * TCC
            xs, zsil = xs_hist[j], zsil_hist[j]
            dts, dtx = dts_hist[j], dtx_hist[j]
            # ---- C: selective scan over n --------------------------------
            pys = [[psY.tile([128, 512], F32, tag="py", name=f"pys_{j}_{h2}_{q2}")
                    for q2 in range(2)] for h2 in range(NDH)]
            for n in range(DS):
                bbc = bcp.tile([128, TCC], BF16, tag="bbc")
                cbc = bcp.tile([128, TCC], BF16, tag="cbc")
                nc.sync.dma_start(
                    out=bbc, in_=bass.AP(tensor=d_bc[:, :].tensor, offset=n * T + t0,
                                         ap=[[0, 128], [1, TCC]]))
                nc.sync.dma_start(
                    out=cbc, in_=bass.AP(tensor=d_bc[:, :].tensor,
                                         offset=(DS + n) * T + t0,
                                         ap=[[0, 128], [1, TCC]]))
                for h in range(NDH):
                    dA = work2.tile([128, TCC], F32, tag="dA")
                    nc.scalar.activation(dA, dts[:, h, :], Expf,
                                         bias=0.0, scale=aneg[:, h, n:n + 1])
                    dBx = work2.tile([128, TCC], BF16, tag="dBx")
                    if n in GP_NS:
                        nc.gpsimd.tensor_mul(dBx, dtx[:, h, :], bbc)
                    else:
                        nc.vector.tensor_mul(dBx, dtx[:, h, :], bbc)
                    hts = work2.tile([128, TCC], BF16, tag="hts")
                    init = 0.0 if (t0 % L == 0) else carry[:, h, n:n + 1]
                    nc.vector.tensor_tensor_scan(
                        out=hts, data0=dA, data1=dBx, initial=init,
                        op0=MUL, op1=ADD)
                    if (t0 + TCC) % L != 0:
                        nc.vector.tensor_copy(carry[:, h, n:n + 1], hts[:, TCC - 1:TCC])
                    yp = work2.tile([128, TCC], BF16, tag="yp")
                    nc.vector.tensor_mul(yp, hts, cbc)
                    for qq in range(2):
                        nc.tensor.matmul(pys[h][qq], lhsT=ident,
                                         rhs=yp[:, qq * 512:(qq + 1) * 512],
                                         start=(n == 0), stop=(n == DS - 1))
            # ---- gate + AllToAll input -----------------------------------
            for h in range(NDH):
                for qq in range(2):
                    g1 = work2.tile([128, 512], BF16, tag="g1")
                    nc.vector.scalar_tensor_tensor(
                        out=g1, in0=xs[:, h, qq * 512:(qq + 1) * 512],
                        scalar=dvec[:, h, 0:1], in1=pys[h][qq],
                        op0=MUL, op1=ADD)
                    ys = work2.tile([128, 512], BF16, tag="ys")
                    nc.vector.tensor_mul(ys, g1, zsil[:, h, qq * 512:(qq + 1) * 512])
                    for i2 in range(4):
                        i = qq * 4 + i2
                        nc.sync.dma_start(
                            out=d_a2ai[j, i, h * 128:(h + 1) * 128, :],
                            in_=ys[:, i2 * 128:(i2 + 1) * 128])
            # ---- AllToAll of this chunk ----------------------------------
            nc.gpsimd.collective_compute(
                kind="AllToAll", op=mybir.AluOpType.bypass, replica_groups=groups,
                ins=[d_a2ai[j, :, :, :]], outs=[d_a2ao[j, :, :, :]])
        def emitE(j):
            # ---- E: out_proj on this core's 128 t-rows of chunk j --------
            ykt = work.tile([128, NCORES, NDH, 128], BF16, tag="ykt")
            nc.sync.dma_start(
                out=ykt, in_=bass.AP(tensor=d_a2ao[:, :, :, :].tensor,
                                     offset=j * NCORES * DL * 128,
                                     ap=[[128, 128], [DL * 128, NCORES],
                                         [128 * 128, NDH], [1, 128]]))
            for fh in range(2):
                pos = psE.tile([128, 512], F32, tag="pe")
                for kt in range(2 * NCORES):
                    i, h = kt // 2, kt % 2
                    nc.tensor.matmul(pos, lhsT=ykt[:, i, h, :],
                                     rhs=wout[:, kt, fh * 512:(fh + 1) * 512],
                                     start=(kt == 0), stop=(kt == 2 * NCORES - 1))
                oc = work.tile([128, 512], F32, tag="oc")
                nc.scalar.activation(oc, pos, Ident, bias=0.0, scale=1.0)
                nc.sync.dma_start(
                    out=d_out[j * 128:(j + 1) * 128, fh * 512:(fh + 1) * 512],
                    in_=oc)

        emitA(0)
        for j in range(NTCB):
            emitB(j)
            if j + 1 < NTCB:
                emitA(j + 1)
            emitC(j)
            emitE(j)

    nc.compile()
    return nc


def _host_prep(inputs):
    """Per-core input maps from full inputs (layout prep + bf16 casts only)."""
    hs = np.asarray(inputs["hidden_states"], np.float32)
    wxz = np.asarray(inputs["in_proj_w"], np.float32)
    cw = np.asarray(inputs["conv_w"], np.float32)
    cb = np.asarray(inputs["conv_b"], np.float32)
    xpw = np.asarray(inputs["x_proj_w"], np.float32)
    dpw = np.asarray(inputs["dt_proj_w"], np.float32)
    dpb = np.asarray(inputs["dt_proj_b"], np.float32)
    alog = np.asarray(inputs["A_log"], np.float32)
    dv = np.asarray(inputs["D"], np.float32)
    wo = np.asarray(inputs["out_proj_w"], np.float32)

    hT = np.ascontiguousarray(hs.reshape(T, DM).T).astype(BF)
    woutT = (np.ascontiguousarray(wo.T) * 0.5).reshape(2 * NCORES, 128, DM).astype(BF)
    ident = (np.eye(128, dtype=np.float32) * 0.5).astype(BF)

    in_maps = []
    for i in range(NCORES):
        lo = i * DL
        sl = slice(lo, lo + DL)
        wxzT = np.ascontiguousarray(
            np.concatenate([wxz[sl], wxz[DI + lo:DI + lo + DL]], axis=0).T).astype(BF)
        cdiag = np.zeros((DC, NDH, 128, 128), np.float32)
        for jc in range(DC):
            for h in range(NDH):
                np.fill_diagonal(cdiag[jc, h], cw[lo + h * 128:lo + (h + 1) * 128, jc])
        in_maps.append({
            "hT": hT,
            "wxzT": wxzT,
            "cdiag": cdiag.astype(BF),
            "cbrow": cb[sl].reshape(1, NDH, 128).astype(BF),
            "xprojT": (np.ascontiguousarray(xpw[:, sl].T) * 0.5).reshape(NDH, 128, 96).astype(BF),
            "dtwT": np.ascontiguousarray(dpw[sl].T).astype(BF),
            "dtb": dpb[sl].reshape(NDH, 128, 1),
            "aneg": (-np.exp(alog[sl])).reshape(NDH, 128, DS).astype(np.float32),
            "dvec": (dv[sl] * 0.5).reshape(NDH, 128, 1),
            "woutT": woutT,
            "ident": ident,
        })
    return in_maps


def _run(inputs, trace=False, **kw):
    if "nc" not in _cached:
        _cached["nc"] = _build()
    nc = _cached["nc"]
    in_maps = _host_prep(inputs)
    res = bass_utils.run_bass_kernel_spmd(
        nc, in_maps, core_ids=list(range(NCORES)), trace=trace, **kw)
    # core i's out_slice rows [j*128:(j+1)*128] are global t rows
    # [j*1024 + i*128 : j*1024 + (i+1)*128]
    full = np.zeros((T, DM), np.float32)
    for i in range(NCORES):
        o = res.results[i]["out_slice"]
        for j in range(NTCB):
            full[j * TCC + i * 128: j * TCC + (i + 1) * 128] = \
                o[j * 128:(j + 1) * 128]
    return full.reshape(B, L, DM), res


def kernel(**inputs):
    out, _ = _run(inputs, trace=False)
    return out


# revision 9
# speedup vs baseline: 1.0656x; 1.0388x over previous
"""Trainium2 Bass kernel for a dense Mamba (selective-scan) block, SPMD over 8 NeuronCores.

Sharding: tensor-parallel over d_inner (2048 -> 256 channels/core), fully
pipelined over 8 time-chunks of 1024 (b,l) positions. Per chunk: in_proj
(bf16 matmul, Silu(z) fused on Act) -> depthwise causal conv via 4 diagonal
matmuls w/ PSUM accumulation + fused Silu -> x_proj partials DMAed straight
from PSUM -> per-chunk AllReduce (393KB) -> dt_proj + fused Softplus ->
selective scan per (chunk, state n): dA = Exp(dt*A[:,n]) (Act), dBx = dtx *
broadcast(B_n) (DVE bf16 2x / partial GpSimd offload), hardware
tensor_tensor_scan (fp32 state), y_n = h * broadcast(C_n), n-reduction via
identity-matmul PSUM accumulation -> gate = (xs*D + psum) via
scalar_tensor_tensor reading PSUM + silu(z) multiply -> per-chunk AllToAll
(512KB) -> out_proj on 128 t-rows with resident weights -> interleaved
per-core output blocks, host re-interleave.

Shapes hardcoded for: B=2, L=4096, d_model=1024, d_inner=2048, d_state=16,
d_conv=4, dt_rank=64, f32 I/O.
"""
import numpy as np
import ml_dtypes
from contextlib import ExitStack

import concourse.bass as bass
import concourse.bacc as bacc
import concourse.tile as tile
from concourse import mybir
from concourse import bass_utils

BF = ml_dtypes.bfloat16
F32 = mybir.dt.float32
BF16 = mybir.dt.bfloat16

NCORES = 8
B, L, DM = 2, 4096, 1024
DI, DS, DC, DTR = 2048, 16, 4, 64
DL = DI // NCORES          # 256 local channels
NDH = DL // 128            # 2 d-half tiles
T = B * L                  # 8192 flattened (b, l)
TCC = 1024                 # pipeline chunk
NTCB = T // TCC            # 8
TCA = 512                  # in_proj sub-chunk
GP_NS = ()         # dBx muls offloaded to gpsimd for these n

_cached = {}


def _build():
    nc = bacc.Bacc("TRN2", target_bir_lowering=False, num_devices=NCORES)

    # ---- I/O -------------------------------------------------------------
    d_hT = nc.dram_tensor("hT", (DM, T), BF16, kind="ExternalInput")
    d_wxzT = nc.dram_tensor("wxzT", (DM, 2 * DL), BF16, kind="ExternalInput")
    d_cdiag = nc.dram_tensor("cdiag", (DC, NDH, 128, 128), BF16, kind="ExternalInput")
    d_cbrow = nc.dram_tensor("cbrow", (1, NDH, 128), BF16, kind="ExternalInput")
    d_xprojT = nc.dram_tensor("xprojT", (NDH, 128, DTR + 2 * DS), BF16, kind="ExternalInput")
    d_dtwT = nc.dram_tensor("dtwT", (DTR, DL), BF16, kind="ExternalInput")
    d_dtb = nc.dram_tensor("dtb", (NDH, 128, 1), F32, kind="ExternalInput")
    d_aneg = nc.dram_tensor("aneg", (NDH, 128, DS), F32, kind="ExternalInput")
    d_dvec = nc.dram_tensor("dvec", (NDH, 128, 1), F32, kind="ExternalInput")
    d_woutT = nc.dram_tensor("woutT", (2 * NCORES, 128, DM), BF16, kind="ExternalInput")
    d_ident = nc.dram_tensor("ident", (128, 128), BF16, kind="ExternalInput")
    d_out = nc.dram_tensor("out_slice", (NTCB * 128, DM), F32, kind="ExternalOutput")

    # ---- internal DRAM ---------------------------------------------------
    d_xdp = nc.dram_tensor("xdp", (NTCB, DTR + 2 * DS, TCC), BF16, kind="Internal")
    d_xd = nc.dram_tensor("xd", (NTCB, DTR + 2 * DS, TCC), BF16, kind="Internal",
                          addr_space="Shared")
    d_bc = nc.dram_tensor("bcrows", (2 * DS, T), BF16, kind="Internal")
    d_a2ai = nc.dram_tensor("a2ai", (NTCB, NCORES, DL, 128), BF16, kind="Internal")
    d_a2ao = nc.dram_tensor("a2ao", (NTCB, NCORES, DL, 128), BF16, kind="Internal")

    groups = [list(range(NCORES))]
    Ident = mybir.ActivationFunctionType.Identity
    Tanh = mybir.ActivationFunctionType.Tanh
    Lnf = mybir.ActivationFunctionType.Ln
    Expf = mybir.ActivationFunctionType.Exp
    MUL, ADD = mybir.AluOpType.mult, mybir.AluOpType.add

    with tile.TileContext(nc) as tc, ExitStack() as ctx:
        consts = ctx.enter_context(tc.tile_pool(name="consts", bufs=1))
        arena = ctx.enter_context(tc.tile_pool(name="arena", bufs=2))
        arena3 = ctx.enter_context(tc.tile_pool(name="arena3", bufs=3))
        work = ctx.enter_context(tc.tile_pool(name="work", bufs=2))
        work2 = ctx.enter_context(tc.tile_pool(name="work2", bufs=3))
        bcp = ctx.enter_context(tc.tile_pool(name="bcp", bufs=4))
        psA = ctx.enter_context(tc.tile_pool(name="psA", bufs=2, space="PSUM"))
        psY = ctx.enter_context(tc.tile_pool(name="psY", bufs=4, space="PSUM"))
        psE = ctx.enter_context(tc.tile_pool(name="psE", bufs=2, space="PSUM"))

        # ---- constants ----------------------------------------------------
        wxz = consts.tile([128, 8, 2 * DL], BF16, tag="wxz")
        for k8 in range(8):
            nc.sync.dma_start(
                out=wxz[:, k8, :],
                in_=bass.AP(tensor=d_wxzT[:, :].tensor, offset=k8 * 128 * 2 * DL,
                            ap=[[2 * DL, 128], [1, 2 * DL]]))
        cdg = consts.tile([128, DC, NDH, 128], BF16, tag="cdg")
        nc.sync.dma_start(
            out=cdg, in_=bass.AP(tensor=d_cdiag[:, :, :, :].tensor, offset=0,
                                 ap=[[128, 128], [NDH * 128 * 128, DC], [128 * 128, NDH], [1, 128]]))
        cbrow = consts.tile([1, NDH, 128], BF16, tag="cbrow")
        nc.sync.dma_start(out=cbrow, in_=d_cbrow[:, :, :])
        onesr = consts.tile([1, TCA], BF16, tag="onesr")
        nc.gpsimd.memset(onesr, 1.0)
        xprj = consts.tile([128, NDH, DTR + 2 * DS], BF16, tag="xprj")
        nc.sync.dma_start(out=xprj, in_=d_xprojT[:, :, :].rearrange("h p m -> p h m"))
        dtw = consts.tile([DTR, DL], BF16, tag="dtw")
        nc.sync.dma_start(out=dtw, in_=d_dtwT[:, :])
        dtb = consts.tile([128, NDH, 1], F32, tag="dtb")
        nc.sync.dma_start(out=dtb, in_=d_dtb[:, :, :].rearrange("h p one -> p h one"))
        aneg = consts.tile([128, NDH, DS], F32, tag="aneg")
        nc.sync.dma_start(out=aneg, in_=d_aneg[:, :, :].rearrange("h p n -> p h n"))
        dvec = consts.tile([128, NDH, 1], F32, tag="dvec")
        nc.sync.dma_start(out=dvec, in_=d_dvec[:, :, :].rearrange("h p one -> p h one"))
        wout = consts.tile([128, 2 * NCORES, DM], BF16, tag="wout")
        for kt in range(2 * NCORES):
            nc.sync.dma_start(
                out=wout[:, kt, :],
                in_=bass.AP(tensor=d_woutT[:, :, :].tensor, offset=kt * 128 * DM,
                            ap=[[DM, 128], [1, DM]]))
        ident = consts.tile([128, 128], BF16, tag="ident")
        nc.sync.dma_start(out=ident, in_=d_ident[:, :])
        carry = consts.tile([128, NDH, DS], F32, tag="carry")

        xpad_hist = [None] * (NTCB + 1)
        zsil_hist = [None] * NTCB
        xs_hist = [None] * NTCB
        dts_hist = [None] * NTCB
        dtx_hist = [None] * NTCB

        def emitA(j):
            t0 = j * TCC
            xpad_prev = xpad_hist[j - 1] if j > 0 else None
            # ---- A: in_proj (+ fused Silu for z) -------------------------
            xpad = arena3.tile([128, NDH, 1027], BF16, tag="xpad")
            zsil = arena3.tile([128, NDH, TCC], BF16, tag="zsil")
            xs = arena3.tile([128, NDH, TCC], BF16, tag="xs")
            if j % (L // TCC) == 0:
                nc.vector.memset(xpad[:, 0, 0:3], 0.0)
                nc.vector.memset(xpad[:, 1, 0:3], 0.0)
            else:
                nc.vector.tensor_copy(xpad[:, 0, 0:3], xpad_prev[:, 0, 1024:1027])
                nc.vector.tensor_copy(xpad[:, 1, 0:3], xpad_prev[:, 1, 1024:1027])
            for s in range(2):
                ts0 = t0 + s * TCA
                ht = work.tile([128, 8, TCA], BF16, tag="ht")
                nc.sync.dma_start(
                    out=ht,
                    in_=bass.AP(tensor=d_hT[:, :].tensor, offset=ts0,
                                ap=[[T, 128], [128 * T, 8], [1, TCA]]))
                for m in range(4):  # 0,1: x halves; 2,3: z halves
                    pxz = psA.tile([128, TCA], F32, tag="ps")
                    for k in range(8):
                        nc.tensor.matmul(pxz, lhsT=wxz[:, k, m * 128:(m + 1) * 128],
                                         rhs=ht[:, k, :], start=(k == 0), stop=(k == 7))
                    if m < 2:
                        nc.scalar.activation(
                            xpad[:, m, 3 + s * TCA: 3 + s * TCA + TCA], pxz,
                            Ident, bias=0.0, scale=1.0)
                    else:
                        th = work.tile([128, TCA], BF16, tag="th")
                        nc.scalar.activation(th, pxz, Tanh, bias=0.0, scale=0.5)
                        nc.vector.scalar_tensor_tensor(
                            out=zsil[:, m - 2, s * TCA: s * TCA + TCA],
                            in0=th, scalar=1.0, in1=pxz, op0=ADD, op1=MUL)
            # ---- conv (4 diag matmuls) + fused Silu ----------------------
            for h in range(NDH):
                for s in range(2):
                    l0s = s * TCA
                    pc = psA.tile([128, TCA], F32, tag="ps")
                    nc.tensor.matmul(pc, lhsT=cbrow[:, h, :], rhs=onesr[:, :],
                                     start=True, stop=False)
                    for jj in range(DC):
                        nc.tensor.matmul(pc, lhsT=cdg[:, jj, h, :],
                                         rhs=xpad[:, h, l0s + jj: l0s + jj + TCA],
                                         start=False, stop=(jj == DC - 1))
                    th = work.tile([128, TCA], BF16, tag="th")
                    nc.scalar.activation(th, pc, Tanh, bias=0.0, scale=0.5)
                    nc.vector.scalar_tensor_tensor(
                        out=xs[:, h, l0s:l0s + TCA],
                        in0=th, scalar=1.0, in1=pc, op0=ADD, op1=MUL)
            # ---- x_proj partial -> DMA straight from PSUM ----------------
            for s in range(2):
                pxp = psA.tile([128, TCA], F32, tag="ps")
                for h in range(NDH):
                    nc.tensor.matmul(pxp[0:96, :], lhsT=xprj[:, h, :],
                                     rhs=xs[:, h, s * TCA:(s + 1) * TCA],
                                     start=(h == 0), stop=(h == NDH - 1))
                xpt = work.tile([96, TCA], BF16, tag="xpt")
                nc.scalar.activation(xpt, pxp[0:96, :], Ident, bias=0.0, scale=1.0)
                nc.sync.dma_start(out=d_xdp[j, :, s * TCA:(s + 1) * TCA], in_=xpt)
            # ---- AllReduce of this chunk's x_dbl partials ----------------
            nc.gpsimd.collective_compute(
                kind="AllReduce", op=mybir.AluOpType.add, replica_groups=groups,
                ins=[d_xdp[j, :, :]], outs=[d_xd[j, :, :]])
            xpad_hist[j] = xpad
            zsil_hist[j] = zsil
            xs_hist[j] = xs

        def emitB(j):
            t0 = j * TCC
            xs = xs_hist[j]
            # ---- B: dt_proj + fused Softplus; dtx; B/C rows --------------
            dts = arena.tile([128, NDH, TCC], BF16, tag="dts")
            dtx = arena.tile([128, NDH, TCC], BF16, tag="dtx")
            xdb = work.tile([96, TCC], BF16, tag="xdb")
            nc.sync.dma_start(out=xdb[:, 0:TCA], in_=d_xd[j, :, 0:TCA])
            nc.sync.dma_start(out=xdb[:, TCA:TCC], in_=d_xd[j, :, TCA:TCC])
            nc.sync.dma_start(out=d_bc[:, t0:t0 + TCC], in_=xdb[DTR:DTR + 2 * DS, :])
            for h in range(NDH):
                for s in range(2):
                    pdt = psA.tile([128, TCA], F32, tag="ps")
                    nc.tensor.matmul(pdt, lhsT=dtw[:, h * 128:(h + 1) * 128],
                                     rhs=xdb[0:DTR, s * TCA:(s + 1) * TCA],
                                     start=True, stop=True)
                    spe = work.tile([128, TCA], F32, tag="spe")
                    nc.scalar.activation(spe, pdt, Expf,
                                         bias=dtb[:, h, 0:1], scale=1.0)
                    nc.scalar.activation(dts[:, h, s * TCA:(s + 1) * TCA], spe,
                                         Lnf, bias=1.0, scale=1.0)
                nc.vector.tensor_mul(dtx[:, h, :], dts[:, h, :], xs[:, h, :])
            dts_hist[j] = dts
            dtx_hist[j] = dtx

        def emitC(j):
            t0 = j * TCC
            xs, zsil = xs_hist[j], zsil_hist[j]
            dts, dtx = dts_hist[j], dtx_hist[j]
            # ---- C: selective scan over n --------------------------------
            pys = [[psY.tile([128, 512], F32, tag="py", name=f"pys_{j}_{h2}_{q2}")
                    for q2 in range(2)] for h2 in range(NDH)]
            for n in range(DS):
                bbc = bcp.tile([128, TCC], BF16, tag="bbc")
                cbc = bcp.tile([128, TCC], BF16, tag="cbc")
                nc.sync.dma_start(
                    out=bbc, in_=bass.AP(tensor=d_bc[:, :].tensor, offset=n * T + t0,
                                         ap=[[0, 128], [1, TCC]]))
                nc.sync.dma_start(
                    out=cbc, in_=bass.AP(tensor=d_bc[:, :].tensor,
                                         offset=(DS + n) * T + t0,
                                         ap=[[0, 128], [1, TCC]]))
                for h in range(NDH):
                    dA = work2.tile([128, TCC], F32, tag="dA")
                    nc.scalar.activation(dA, dts[:, h, :], Expf,
                                         bias=0.0, scale=aneg[:, h, n:n + 1])
                    dBx = work2.tile([128, TCC], BF16, tag="dBx")
                    if n in GP_NS:
                        nc.gpsimd.tensor_mul(dBx, dtx[:, h, :], bbc)
                    else:
                        nc.vector.tensor_mul(dBx, dtx[:, h, :], bbc)
                    hts = work2.tile([128, TCC], BF16, tag="hts")
                    init = 0.0 if (t0 % L == 0) else carry[:, h, n:n + 1]
                    nc.vector.tensor_tensor_scan(
                        out=hts, data0=dA, data1=dBx, initial=init,
                        op0=MUL, op1=ADD)
                    if (t0 + TCC) % L != 0:
                        nc.vector.tensor_copy(carry[:, h, n:n + 1], hts[:, TCC - 1:TCC])
                    yp = work2.tile([128, TCC], BF16, tag="yp")
                    nc.vector.tensor_mul(yp, hts, cbc)
                    for qq in range(2):
                        nc.tensor.matmul(pys[h][qq], lhsT=ident,
                                         rhs=yp[:, qq * 512:(qq + 1) * 512],
                                         start=(n == 0), stop=(n == DS - 1))
            # ---- gate + AllToAll input -----------------------------------
            for h in range(NDH):
                for qq in range(2):
                    g1 = work2.tile([128, 512], BF16, tag="g1")
                    nc.vector.scalar_tensor_tensor(
                        out=g1, in0=xs[:, h, qq * 512:(qq + 1) * 512],
                        scalar=dvec[:, h, 0:1], in1=pys[h][qq],
                        op0=MUL, op1=ADD)
                    ys = work2.tile([128, 512], BF16, tag="ys")
                    nc.vector.tensor_mul(ys, g1, zsil[:, h, qq * 512:(qq + 1) * 512])
                    for i2 in range(4):
                        i = qq * 4 + i2
                        nc.sync.dma_start(
                            out=d_a2ai[j, i, h * 128:(h + 1) * 128, :],
                            in_=ys[:, i2 * 128:(i2 + 1) * 128])
            # ---- AllToAll of this chunk ----------------------------------
            nc.gpsimd.collective_compute(
                kind="AllToAll", op=mybir.AluOpType.bypass, replica_groups=groups,
                ins=[d_a2ai[j, :, :, :]], outs=[d_a2ao[j, :, :, :]])
        def emitE(j):
            # ---- E: out_proj on this core's 128 t-rows of chunk j --------
            ykt = work.tile([128, NCORES, NDH, 128], BF16, tag="ykt")
            nc.sync.dma_start(
                out=ykt, in_=bass.AP(tensor=d_a2ao[:, :, :, :].tensor,
                                     offset=j * NCORES * DL * 128,
                                     ap=[[128, 128], [DL * 128, NCORES],
                                         [128 * 128, NDH], [1, 128]]))
            for fh in range(2):
                pos = psE.tile([128, 512], F32, tag="pe")
                for kt in range(2 * NCORES):
                    i, h = kt // 2, kt % 2
                    nc.tensor.matmul(pos, lhsT=ykt[:, i, h, :],
                                     rhs=wout[:, kt, fh * 512:(fh + 1) * 512],
                                     start=(kt == 0), stop=(kt == 2 * NCORES - 1))
                oc = work.tile([128, 512], F32, tag="oc")
                nc.scalar.activation(oc, pos, Ident, bias=0.0, scale=1.0)
                nc.sync.dma_start(
                    out=d_out[j * 128:(j + 1) * 128, fh * 512:(fh + 1) * 512],
                    in_=oc)

        emitA(0)
        emitA(1)
        for j in range(NTCB):
            emitB(j)
            if j + 2 < NTCB:
                emitA(j + 2)
            emitC(j)
            emitE(j)

    nc.compile()
    return nc


def _host_prep(inputs):
    """Per-core input maps from full inputs (layout prep + bf16 casts only)."""
    hs = np.asarray(inputs["hidden_states"], np.float32)
    wxz = np.asarray(inputs["in_proj_w"], np.float32)
    cw = np.asarray(inputs["conv_w"], np.float32)
    cb = np.asarray(inputs["conv_b"], np.float32)
    xpw = np.asarray(inputs["x_proj_w"], np.float32)
    dpw = np.asarray(inputs["dt_proj_w"], np.float32)
    dpb = np.asarray(inputs["dt_proj_b"], np.float32)
    alog = np.asarray(inputs["A_log"], np.float32)
    dv = np.asarray(inputs["D"], np.float32)
    wo = np.asarray(inputs["out_proj_w"], np.float32)

    hT = np.ascontiguousarray(hs.reshape(T, DM).T).astype(BF)
    woutT = (np.ascontiguousarray(wo.T) * 0.5).reshape(2 * NCORES, 128, DM).astype(BF)
    ident = (np.eye(128, dtype=np.float32) * 0.5).astype(BF)

    in_maps = []
    for i in range(NCORES):
        lo = i * DL
        sl = slice(lo, lo + DL)
        wxzT = np.ascontiguousarray(
            np.concatenate([wxz[sl], wxz[DI + lo:DI + lo + DL]], axis=0).T).astype(BF)
        cdiag = np.zeros((DC, NDH, 128, 128), np.float32)
        for jc in range(DC):
            for h in range(NDH):
                np.fill_diagonal(cdiag[jc, h], cw[lo + h * 128:lo + (h + 1) * 128, jc])
        in_maps.append({
            "hT": hT,
            "wxzT": wxzT,
            "cdiag": cdiag.astype(BF),
            "cbrow": cb[sl].reshape(1, NDH, 128).astype(BF),
            "xprojT": (np.ascontiguousarray(xpw[:, sl].T) * 0.5).reshape(NDH, 128, 96).astype(BF),
            "dtwT": np.ascontiguousarray(dpw[sl].T).astype(BF),
            "dtb": dpb[sl].reshape(NDH, 128, 1),
            "aneg": (-np.exp(alog[sl])).reshape(NDH, 128, DS).astype(np.float32),
            "dvec": (dv[sl] * 0.5).reshape(NDH, 128, 1),
            "woutT": woutT,
            "ident": ident,
        })
    return in_maps


def _run(inputs, trace=False, **kw):
    if "nc" not in _cached:
        _cached["nc"] = _build()
    nc = _cached["nc"]
    in_maps = _host_prep(inputs)
    res = bass_utils.run_bass_kernel_spmd(
        nc, in_maps, core_ids=list(range(NCORES)), trace=trace, **kw)
    # core i's out_slice rows [j*128:(j+1)*128] are global t rows
    # [j*1024 + i*128 : j*1024 + (i+1)*128]
    full = np.zeros((T, DM), np.float32)
    for i in range(NCORES):
        o = res.results[i]["out_slice"]
        for j in range(NTCB):
            full[j * TCC + i * 128: j * TCC + (i + 1) * 128] = \
                o[j * 128:(j + 1) * 128]
    return full.reshape(B, L, DM), res


def kernel(**inputs):
    out, _ = _run(inputs, trace=False)
    return out
